# revision 3
# baseline (speedup 1.0000x reference)
"""ColorHistogramLoss (soft histogram EMD) on 8 Trainium2 NeuronCores.

Strategy: pure data parallel over batch (B=8 -> one batch element per core).
Each core computes, for its 3 channels x {pred, target}, the 64-bin soft
(Gaussian-weighted) histogram of its 384x384 image:

    hist[j] = sum_px exp(-(x_px - c_j)^2 / denom)

The work is split across TWO engines running concurrently:

* ACT (scalar) engine: Derivative_Erf(scale*x + bias) with fused accum_out
  free-dim reduction, per-partition bias selecting 8 bins per instruction
  (the input tile is replicated 8x across partition blocks host-side).
  One instruction covers one bin-octet of one image at FD=9216.

* DVE (vector) engine: a Schraudolph bit-trick exp pipeline for a subset of
  bin-octets, 4 passes per octet-task at bf16/2x-4x perf modes:
    P1  d  = bf16(u*K1 - c~[p])            tensor_scalar dual-op, 4x
    P2  s  = bf16(-d*d)                    scalar_tensor_tensor, 2x
    P3  q  = int32(max(s, -B) + B)         tensor_scalar dual-op int32-out
    P4  bitcast fp32(q) -> accum_out       tensor_scalar bypass w/ accum
  int32 bits B - M*(u-j)^2 reinterpreted as fp32 give ~exp(-kappa(u-j)^2)
  with a ~3% deterministic bias, corrected per-bin on the host (GAMMA64,
  computed offline over a uniform u-grid simulating the exact pipeline).

Input tiles are bf16 (halves DMA); both engines read the same tile.
Per-partition partial sums land in hacc[128, 48]; one PE matmul against an
8-column block selector reduces the 16 rows of each block, giving all 384
histogram values as [48, 8] in PSUM.  The tiny tail (normalize, cumsum,
|diff|, mean) runs on host in float64.

Measured notes from the ACT-only baseline: ACT instruction ~7.2us per
octet-image; ACT is rate-limited at 1 elem/lane/cycle dtype-independent, so
the only speedup lever is offloading octets to the otherwise-idle DVE.
An on-device step-0 replicating DMA was tried previously and intermittently
corrupts results / wedges the device - do not reintroduce it.
"""

import functools
import math

import numpy as np

N_CORES = 8
NUM_BINS = 64
B, C, H, W = 8, 3, 384, 384
HW = H * W
N_UNITS = 2 * C                 # (channel, pred/target) images per core
N_OCT = NUM_BINS // 8           # 8 bin-octets per image
FREE = HW // 16                 # channel image as [16, 9216]
DENOM = 2.0 * (1.0 / 64.0) ** 2 + 1e-7
SCALE = 1.0 / math.sqrt(DENOM)
DERF_SCALE = math.sqrt(math.pi) / 2.0  # Derivative_Erf = 2/sqrt(pi) * exp(-u^2)

# Schraudolph constants: bits = B_EXP - M_SCH * (u*63 - j)^2
KAPPA = 1.0 / (63.0 ** 2 * DENOM)
M_SCH = KAPPA * math.log2(math.e) * (1 << 23)
SQM = math.sqrt(M_SCH)
K1 = 63.0 * SQM                 # d = x*K1 - j*SQM
B_EXP = float(127 << 23)
SMIN = -B_EXP

# Which bin-octets each unit's DVE handles (rest go to ACT).  Tuned so both
# engines finish together: ACT ~7.2us/octet, DVE ~17us/octet.
DVE_OCTS = ([0, 6, 7], [0, 6, 7], [0, 7], [0, 7], [0, 7], [0, 7])

# Host-side per-bin multiplicative correction for the DVE octets:
# exact-Gaussian integral / simulated-pipeline integral over uniform u.
GAMMA64 = np.array([
    0.97004448, 0.97084343, 0.96982991, 0.96963145, 0.96969907, 0.96978281, 0.96965175, 0.96959383,
    0.96971529, 0.96988192, 0.96998091, 0.96998932, 0.96964085, 0.96968675, 0.96961145, 0.96939361,
    0.96948104, 0.96981380, 0.96981607, 0.97013160, 0.96994397, 0.96997168, 0.96991644, 0.96904932,
    0.96964121, 0.96935956, 0.97013152, 0.97012283, 0.96958524, 0.96973371, 0.96832287, 0.96862380,
    0.96992517, 0.96952056, 0.96992766, 0.96950274, 0.97088229, 0.97060633, 0.97035799, 0.97032050,
    0.96940238, 0.97032061, 0.97019507, 0.97015197, 0.97016509, 0.96954148, 0.96902073, 0.96838173,
    0.96891450, 0.96876182, 0.96921661, 0.96987129, 0.97062693, 0.97112053, 0.97070180, 0.97039496,
    0.97015796, 0.97007996, 0.97063135, 0.96961566, 0.96924145, 0.97010983, 0.97213848, 0.96931856,
], dtype=np.float64)

ACT_OCTS = tuple(
    tuple(o for o in range(N_OCT) if o not in DVE_OCTS[u]) for u in range(N_UNITS)
)
N_ACT_PER_UNIT = tuple(len(a) for a in ACT_OCTS)
N_DVE_PER_UNIT = tuple(len(d) for d in DVE_OCTS)
TOT_ACT = sum(N_ACT_PER_UNIT)
TOT_DVE = sum(N_DVE_PER_UNIT)


def _cum_units(counts, n_units_done, r_len):
    """Sem count after the first `n_units_done` global units are consumed."""
    full, rem = divmod(n_units_done, r_len)
    return full * sum(counts) + sum(counts[:rem])


def _build_program(R=1):
    import concourse.bass as bass
    import concourse.mybir as mybir

    nc = bass.Bass()
    xs = [
        nc.dram_tensor(f"x{u}", [128, FREE], mybir.dt.bfloat16, kind="ExternalInput")
        for u in range(N_UNITS)
    ]
    cst = nc.dram_tensor("consts", [128, 24], mybir.dt.float32, kind="ExternalInput")
    hist_out = nc.dram_tensor(
        "hist", [N_UNITS * N_OCT, 8], mybir.dt.float32, kind="ExternalOutput"
    )

    with (
        nc.sbuf_tensor("xt0", [128, FREE], mybir.dt.bfloat16) as xt0,
        nc.sbuf_tensor("xt1", [128, FREE], mybir.dt.bfloat16) as xt1,
        nc.sbuf_tensor("xt2", [128, FREE], mybir.dt.bfloat16) as xt2,
        nc.sbuf_tensor("cstt", [128, 24], mybir.dt.float32) as cstt,
        nc.sbuf_tensor("wdump", [128, FREE], mybir.dt.bfloat16) as wdump,
        nc.sbuf_tensor("wscr", [128, 8], mybir.dt.float32) as wscr,
        nc.sbuf_tensor("dbuf", [128, FREE], mybir.dt.bfloat16) as dbuf,
        nc.sbuf_tensor("sbufs", [128, FREE], mybir.dt.bfloat16) as sbufs,
        nc.sbuf_tensor("qbuf", [128, FREE], mybir.dt.int32) as qbuf,
        nc.sbuf_tensor("hacc", [128, N_UNITS * N_OCT], mybir.dt.float32) as hacc,
        nc.sbuf_tensor("ho", [N_UNITS * N_OCT, 8], mybir.dt.float32) as ho,
        nc.psum_tensor("ph", [N_UNITS * N_OCT, 8], mybir.dt.float32) as ph,
        nc.semaphore("sem_c") as sem_c,
        nc.semaphore("sem_x0") as sem_x0,
        nc.semaphore("sem_x1") as sem_x1,
        nc.semaphore("sem_x2") as sem_x2,
        nc.semaphore("act_sem") as act_sem,
        nc.semaphore("dve_sem") as dve_sem,
        nc.semaphore("pe_sem") as pe_sem,
        nc.semaphore("cp_sem") as cp_sem,
        nc.Block() as block,
    ):
        slots = [xt0, xt1, xt2]
        xsems = [sem_x0, sem_x1, sem_x2]

        @block.sync
        def _(sync):
            sync.dma_start(out=cstt[:], in_=cst[:]).then_inc(sem_c, 16)
            for r in range(R):
                for u in range(N_UNITS):
                    g = r * N_UNITS + u
                    if g >= 3:
                        sync.wait_ge(act_sem, _cum_units(N_ACT_PER_UNIT, g - 2, N_UNITS))
                        sync.wait_ge(dve_sem, _cum_units(N_DVE_PER_UNIT, g - 2, N_UNITS))
                    sync.dma_start(out=slots[g % 3][:], in_=xs[u][:]).then_inc(
                        xsems[g % 3], 16
                    )
                sync.wait_ge(cp_sem, r + 1)
                sync.dma_start(out=hist_out[:], in_=ho[:]).then_inc(sem_c, 16)

        @block.scalar
        def _(scalar):
            # dummy activation on scratch: pulls the ACT table load (~2.7us)
            # forward so it overlaps with the input DMAs
            scalar.activation(
                wscr[0:128, 0:1], wscr[0:128, 1:2],
                mybir.ActivationFunctionType.Derivative_Erf,
                bias=wscr[:, 2:3], scale=1.0,
            )
            scalar.wait_ge(sem_c, 16)
            for r in range(R):
                for u in range(N_UNITS):
                    g = r * N_UNITS + u
                    slot = g % 3
                    scalar.wait_ge(xsems[slot], 16 * (g // 3 + 1))
                    for o in ACT_OCTS[u]:
                        # partition block k (rows 16k..16k+15) evaluates bin 8o+k
                        scalar.activation(
                            wdump[:],
                            slots[slot][:],
                            mybir.ActivationFunctionType.Derivative_Erf,
                            bias=cstt[:, o : o + 1],
                            scale=float(SCALE),
                            accum_out=hacc[:, N_OCT * u + o : N_OCT * u + o + 1],
                        ).then_inc(act_sem, 1)

        @block.vector
        def _(vector):
            for r in range(R):
                for u in range(N_UNITS):
                    g = r * N_UNITS + u
                    slot = g % 3
                    if DVE_OCTS[u]:
                        vector.wait_ge(xsems[slot], 16 * (g // 3 + 1))
                    for o in DVE_OCTS[u]:
                        col = N_OCT * u + o
                        vector.tensor_scalar(
                            dbuf[:], slots[slot][:],
                            float(K1), cstt[:, 16 + o : 17 + o],
                            mybir.AluOpType.mult, mybir.AluOpType.subtract,
                        )
                        vector.scalar_tensor_tensor(
                            sbufs[:], dbuf[:], -1.0, dbuf[:],
                            mybir.AluOpType.mult, mybir.AluOpType.mult,
                        )
                        vector.tensor_scalar(
                            qbuf[:], sbufs[:],
                            float(SMIN), float(B_EXP),
                            mybir.AluOpType.max, mybir.AluOpType.add,
                        )
                        vector.tensor_scalar(
                            dbuf[:], qbuf[:].bitcast(mybir.dt.float32),
                            1.0, 0.0,
                            mybir.AluOpType.mult, mybir.AluOpType.add,
                            accum_out=hacc[:, col : col + 1],
                        ).then_inc(dve_sem, 1)
                # final reduce + copy-out for this repeat
                vector.wait_ge(pe_sem, r + 1)
                vector.tensor_copy(ho[:, :], ph[:, :]).then_inc(cp_sem, 1)

        @block.tensor
        def _(tensor):
            for r in range(R):
                tensor.wait_ge(act_sem, TOT_ACT * (r + 1))
                tensor.wait_ge(dve_sem, TOT_DVE * (r + 1))
                # ph[col, k] = sum_p hacc[p, col] * sel[p, k]  (sel: p//16 == k)
                tensor.matmul(
                    ph[0 : N_UNITS * N_OCT, 0:8],
                    hacc[:, :],
                    cstt[:, 8:16],
                    start=True,
                    stop=True,
                ).then_inc(pe_sem, 1)

    return nc


def _make_consts():
    centers = np.linspace(0.0, 1.0, NUM_BINS, dtype=np.float32)
    bias = (-centers.astype(np.float64) * SCALE).astype(np.float32)
    cst = np.zeros((128, 24), dtype=np.float32)
    p = np.arange(128)
    for o in range(N_OCT):
        cst[:, o] = bias[8 * o + p // 16]      # ACT per-partition bias
        cst[:, 16 + o] = ((8 * o + p // 16) * SQM).astype(np.float32)  # DVE c~
    for k in range(8):
        cst[p // 16 == k, 8 + k] = 1.0         # block selector for the PE reduce
    return cst


@functools.lru_cache(maxsize=2)
def _get_runner(R=1):
    """Compile the SPMD program once; return a callable list[in_map] -> list[out_map]."""
    import jax
    from jax.experimental.shard_map import shard_map
    from jax.sharding import Mesh, PartitionSpec

    from concourse import mybir
    from concourse.bass2jax import (
        _bass_exec_p,
        install_neuronx_cc_hook,
        partition_id_tensor,
    )

    nc = _build_program(R)
    install_neuronx_cc_hook()

    partition_name = (
        nc.partition_id_tensor.name if nc.partition_id_tensor else None
    )
    in_names, out_names, out_avals, zero_outs = [], [], [], []
    for alloc in nc.m.functions[0].allocations:
        if not isinstance(alloc, mybir.MemoryLocationSet):
            continue
        name = alloc.memorylocations[0].name
        if alloc.kind == "ExternalInput":
            if name != partition_name:
                in_names.append(name)
        elif alloc.kind == "ExternalOutput":
            out_names.append(name)
            shape = tuple(alloc.tensor_shape)
            dtype = mybir.dt.np(alloc.dtype)
            out_avals.append(jax.core.ShapedArray(shape, dtype))
            zero_outs.append(np.zeros(shape, dtype))
    n_params = len(in_names)
    n_outs = len(out_avals)
    all_in_names = list(in_names) + list(out_names)
    if partition_name is not None:
        all_in_names.append(partition_name)
    donate = tuple(range(n_params, n_params + n_outs))

    def _body(*args):
        operands = list(args)
        if partition_name is not None:
            operands.append(partition_id_tensor())
        outs = _bass_exec_p.bind(
            *operands,
            out_avals=tuple(out_avals),
            in_names=tuple(all_in_names),
            out_names=tuple(out_names),
            lowering_input_output_aliases=(),
            sim_require_finite=True,
            sim_require_nnan=True,
            nc=nc,
        )
        return tuple(outs)

    devices = jax.devices()[:N_CORES]
    mesh = Mesh(np.asarray(devices), ("core",))
    sharded = jax.jit(
        shard_map(
            _body,
            mesh=mesh,
            in_specs=(PartitionSpec("core"),) * (n_params + n_outs),
            out_specs=(PartitionSpec("core"),) * n_outs,
            check_rep=False,
        ),
        donate_argnums=donate,
        keep_unused=True,
    )

    class Runner:
        def __init__(self):
            self.sharded = sharded
            self.in_names = in_names
            self.out_names = out_names
            self.out_avals = out_avals
            self.zero_outs = zero_outs

        def concat_inputs(self, in_maps):
            return [
                np.concatenate([np.asarray(m[name]) for m in in_maps], axis=0)
                for name in in_names
            ]

        def fresh_zeros(self):
            return [
                np.zeros((N_CORES * z.shape[0], *z.shape[1:]), z.dtype)
                for z in zero_outs
            ]

        def split_outputs(self, out_arrs):
            return [
                {
                    name: np.asarray(out_arrs[i]).reshape(
                        N_CORES, *out_avals[i].shape
                    )[c]
                    for i, name in enumerate(out_names)
                }
                for c in range(N_CORES)
            ]

        def __call__(self, in_maps):
            out_arrs = self.sharded(*self.concat_inputs(in_maps), *self.fresh_zeros())
            return self.split_outputs(out_arrs)

    return Runner()


def _shard_inputs(pred, target):
    import ml_dtypes

    cst = _make_consts()
    maps = []
    for b in range(B):
        m = {"consts": cst}
        for c in range(C):
            for t, src in enumerate((pred, target)):
                u = 2 * c + t
                img = np.ascontiguousarray(src[b, c], dtype=np.float32).reshape(
                    16, FREE
                )
                m[f"x{u}"] = np.tile(img, (8, 1)).astype(ml_dtypes.bfloat16)
        maps.append(m)
    return maps


# Per-(row, k) host-side scale for the [48, 8] device histogram output.
def _make_row_scale():
    s = np.empty((N_UNITS * N_OCT, 8), dtype=np.float64)
    for u in range(N_UNITS):
        for o in range(N_OCT):
            row = N_OCT * u + o
            if o in DVE_OCTS[u]:
                s[row, :] = GAMMA64[8 * o : 8 * o + 8]
            else:
                s[row, :] = DERF_SCALE
    return s


_ROW_SCALE = _make_row_scale()


def _finish_on_host(results):
    total = 0.0
    for b in range(B):
        hist = results[b]["hist"].astype(np.float64) * _ROW_SCALE
        for c in range(C):
            p = hist[N_OCT * (2 * c) : N_OCT * (2 * c) + N_OCT, :].reshape(NUM_BINS)
            t = hist[N_OCT * (2 * c + 1) : N_OCT * (2 * c + 1) + N_OCT, :].reshape(
                NUM_BINS
            )
            pn = p / (p.sum() + 1e-7)
            tn = t / (t.sum() + 1e-7)
            total += np.abs(np.cumsum(pn) - np.cumsum(tn)).sum()
    return np.float32(total / (B * C * NUM_BINS))


def kernel(pred, target):
    pred = np.asarray(pred, dtype=np.float32)
    target = np.asarray(target, dtype=np.float32)
    assert pred.shape == (B, C, H, W) and target.shape == (B, C, H, W)
    run = _get_runner()
    results = run(_shard_inputs(pred, target))
    return np.asarray(_finish_on_host(results), dtype=np.float32)
